# revision 1
# baseline (speedup 1.0000x reference)
"""Trainium2 Bass kernel for the deformed-pixel Gaussian-RBF problem.

Computes, for 65536 pixels and 2048 centers:
    deformation = K_def @ betas                       [N, 2]
    dp          = all_pixels - deformation            [N, 2]
    d2[p, c]    = ||dp[p] - center[c]||^2
    out[p]      = sum_c exp(-d2[p, c] / 2) * alphas[c]

Sharding: pixel axis split row-parallel over 8 NeuronCores (8192 px/core).
K_def is pre-transposed on the host so each core streams [g, pix] tiles with
contiguous rows; centers/alphas/betas are replicated.

Math rearrangement:
    -d2/2 = dp.c - |dp|^2/2 - |c|^2/2
    out[p] = sum_c (alphas[c] * e^{-|c|^2/2}) * exp(dp.c - |dp|^2/2)
The |c|^2 term is folded into the alpha weights on the host (al_eff);
the |dp|^2 term rides in ScalarE's per-partition activation bias.

Per-core device pipeline:
  PE   : deformation^T = betas^T @ K_def^T   (K=128 contractions, fp32r)
         m = dpT^T @ [cx; cy]                (K=2, fp32r)
         bias = sqT^T @ [-1/2; -1/2]         (K=2, N=1, fp32)
  DVE  : dp/dp^2 assembly, fused multiply-reduce of kern * al_eff
  ACT  : kern = exp(m + bias) on [128, ARG_W] PSUM tiles
"""

import numpy as np
from contextlib import ExitStack

N_CORES = 8
N_PIX = 65536
N_CEN = 2048
N_G = 1024
NPC = N_PIX // N_CORES  # pixels per core

# device tiling parameters (full-scale)
PIX_BLK = 512  # pixel block for deformation psum tiles [2, PIX_BLK]
ARG_W = 1024  # centers per arg psum tile [128, ARG_W]


def _build_program(npc, n_cen, n_g, pix_blk, arg_w, kt_bufs=16, arg_bufs=2, reps=1):
    """reps>1 wraps the whole compute body in a hardware loop — used only for
    timing (amortizes the host->device dispatch overhead over many runs)."""
    import concourse.bacc as bacc
    import concourse.tile as tile
    from concourse import mybir

    f32 = mybir.dt.float32
    f32r = mybir.dt.float32r
    AF = mybir.ActivationFunctionType
    OP = mybir.AluOpType

    n_blk = npc // pix_blk
    n_gt = n_g // 128
    tiles_per_blk = pix_blk // 128
    n_half = n_cen // arg_w
    mm_n = min(arg_w, 512)
    mm_per_half = arg_w // mm_n
    n_tiles = npc // 128

    nc = bacc.Bacc(
        "TRN2", target_bir_lowering=False, debug=False, num_devices=N_CORES
    )

    kt = nc.dram_tensor("kt", [n_g, npc], f32r, kind="ExternalInput")
    pxt = nc.dram_tensor("pxt", [2, npc], f32, kind="ExternalInput")
    b2 = nc.dram_tensor("b2", [2, n_cen], f32r, kind="ExternalInput")
    alb = nc.dram_tensor("alb", [128, n_cen], f32, kind="ExternalInput")
    bre = nc.dram_tensor("bre", [128, 2 * n_gt], f32r, kind="ExternalInput")
    mhalf = nc.dram_tensor("mhalf", [2, 1], f32, kind="ExternalInput")
    out = nc.dram_tensor("out", [128, n_tiles], f32, kind="ExternalOutput")

    with tile.TileContext(nc) as tc:
        with ExitStack() as ctx:
            statics = ctx.enter_context(tc.tile_pool(name="statics", bufs=1))
            ktp = ctx.enter_context(tc.tile_pool(name="ktp", bufs=kt_bufs))
            dptp = ctx.enter_context(tc.tile_pool(name="dptp", bufs=3))
            sqtp = ctx.enter_context(tc.tile_pool(name="sqtp", bufs=3))
            kernp = ctx.enter_context(tc.tile_pool(name="kernp", bufs=3))
            junkp = ctx.enter_context(tc.tile_pool(name="junkp", bufs=1))
            partp = ctx.enter_context(tc.tile_pool(name="partp", bufs=4))
            biassp = ctx.enter_context(tc.tile_pool(name="biassp", bufs=4))
            defp = ctx.enter_context(tc.tile_pool(name="defp", bufs=2, space="PSUM"))
            biaspp = ctx.enter_context(
                tc.tile_pool(name="biaspp", bufs=2, space="PSUM")
            )
            argp = ctx.enter_context(
                tc.tile_pool(name="argp", bufs=arg_bufs, space="PSUM")
            )

            pxt_sb = statics.tile([2, npc], f32)
            nc.sync.dma_start(out=pxt_sb[:], in_=pxt[:, :])
            b2_sb = statics.tile([2, n_cen], f32r)
            nc.sync.dma_start(out=b2_sb[:], in_=b2[:, :])
            alb_sb = statics.tile([128, n_cen], f32)
            nc.sync.dma_start(out=alb_sb[:], in_=alb[:, :])
            bre_sb = statics.tile([128, 2 * n_gt], f32r)
            nc.sync.dma_start(out=bre_sb[:], in_=bre[:, :])
            mhalf_sb = statics.tile([2, 1], f32)
            nc.sync.dma_start(out=mhalf_sb[:], in_=mhalf[:, :])
            out_sb = statics.tile([128, n_tiles], f32)

            def body():
                emit_body(
                    nc, tc, mybir,
                    n_blk, pix_blk, n_gt, tiles_per_blk, n_half, mm_n,
                    mm_per_half, arg_w,
                    kt, pxt_sb, b2_sb, alb_sb, bre_sb, mhalf_sb, out_sb,
                    ktp, dptp, sqtp, kernp, junkp, partp, biassp,
                    defp, biaspp, argp,
                )

            if reps == 1:
                body()
            else:
                ET = mybir.EngineType
                with tc.For_i(
                    0, reps, 1,
                    hint_engines=(ET.PE, ET.Activation, ET.DVE, ET.SP, ET.Pool),
                ):
                    body()

            nc.sync.dma_start(out=out[:, :], in_=out_sb[:])

    nc.compile()
    return nc


def emit_body(
    nc, tc, mybir,
    n_blk, pix_blk, n_gt, tiles_per_blk, n_half, mm_n, mm_per_half, arg_w,
    kt, pxt_sb, b2_sb, alb_sb, bre_sb, mhalf_sb, out_sb,
    ktp, dptp, sqtp, kernp, junkp, partp, biassp, defp, biaspp, argp,
):
    f32 = mybir.dt.float32
    f32r = mybir.dt.float32r
    AF = mybir.ActivationFunctionType
    OP = mybir.AluOpType

    for pb in range(n_blk):
                p0 = pb * pix_blk
                # deformation^T for this pixel block: [2, pix_blk] psum
                dpsum = defp.tile([2, pix_blk], f32)
                for g in range(n_gt):
                    kt_t = ktp.tile([128, pix_blk], f32r)
                    nc.sync.dma_start(
                        out=kt_t[:],
                        in_=kt[g * 128 : (g + 1) * 128, p0 : p0 + pix_blk],
                    )
                    nc.tensor.matmul(
                        dpsum[:],
                        bre_sb[:, 2 * g : 2 * g + 2],
                        kt_t[:],
                        start=(g == 0),
                        stop=(g == n_gt - 1),
                    )
                # dp^T = pixels^T - deformation^T ; sq^T = dp^T * dp^T
                # (written as f32r: the arg matmul consumes it at full PE rate)
                dpT = dptp.tile([2, pix_blk], f32r)
                nc.vector.scalar_tensor_tensor(
                    out=dpT[:],
                    in0=dpsum[:],
                    scalar=-1.0,
                    in1=pxt_sb[:, p0 : p0 + pix_blk],
                    op0=OP.mult,
                    op1=OP.add,
                )
                sqT = sqtp.tile([2, pix_blk], f32)
                nc.vector.tensor_tensor(sqT[:], dpT[:], dpT[:], OP.mult)

                for j in range(tiles_per_blk):
                    t = pb * tiles_per_blk + j
                    js = slice(j * 128, (j + 1) * 128)
                    # bias[p] = -|dp[p]|^2 / 2  (K=2, N=1 fp32 matmul)
                    bias_ps = biaspp.tile([128, 1], f32)
                    nc.tensor.matmul(
                        bias_ps[:], sqT[:, js], mhalf_sb[:], start=True, stop=True
                    )
                    bias_sb = biassp.tile([128, 1], f32)
                    nc.vector.tensor_copy(bias_sb[:], bias_ps[:])

                    lhsT = dpT[:, js]
                    parts = []
                    for h in range(n_half):
                        argt = argp.tile([128, arg_w], f32)
                        for q in range(mm_per_half):
                            c0 = h * arg_w + q * mm_n
                            nc.tensor.matmul(
                                argt[:, q * mm_n : (q + 1) * mm_n],
                                lhsT,
                                b2_sb[:, c0 : c0 + mm_n],
                                start=True,
                                stop=True,
                            )
                        kern = kernp.tile([128, arg_w], f32)
                        nc.scalar.activation(
                            kern[:], argt[:], AF.Exp, bias=bias_sb[:]
                        )
                        junk = junkp.tile([128, arg_w], f32)
                        part = partp.tile([128, 1], f32)
                        nc.vector.scalar_tensor_tensor(
                            out=junk[:],
                            in0=kern[:],
                            scalar=1.0,
                            in1=alb_sb[:, h * arg_w : (h + 1) * arg_w],
                            op0=OP.bypass,
                            op1=OP.mult,
                            accum_out=part[:],
                        )
                        parts.append(part)
                    if len(parts) == 2:
                        nc.vector.tensor_tensor(
                            out_sb[:, t : t + 1], parts[0][:], parts[1][:], OP.add
                        )
                    else:
                        acc = parts[0]
                        for extra in parts[1:]:
                            nc.vector.tensor_tensor(acc[:], acc[:], extra[:], OP.add)
                        nc.vector.tensor_copy(out_sb[:, t : t + 1], acc[:])


def _prep_inputs(betas, K_def, all_pixels, all_p_centers, alphas, npc, n_g):
    """Host-side sharding/layout prep. Returns per-core input maps."""
    n_gt = n_g // 128
    n_cores = K_def.shape[0] // npc

    K_T = np.ascontiguousarray(K_def.T.astype(np.float32))  # [n_g, N_PIX]

    cx = all_p_centers[:, 0].astype(np.float32)
    cy = all_p_centers[:, 1].astype(np.float32)
    cen_sq = (
        all_p_centers.astype(np.float64)[:, 0] ** 2
        + all_p_centers.astype(np.float64)[:, 1] ** 2
    )
    n_cen = cx.shape[0]
    b2 = np.stack([cx, cy]).astype(np.float32)
    al_eff = (
        alphas.astype(np.float64).reshape(n_cen) * np.exp(-0.5 * cen_sq)
    ).astype(np.float32)
    alb = np.ascontiguousarray(
        np.broadcast_to(al_eff.reshape(1, n_cen), (128, n_cen))
    )
    bre = np.ascontiguousarray(
        betas.astype(np.float32)
        .reshape(n_gt, 128, 2)
        .transpose(1, 0, 2)
        .reshape(128, 2 * n_gt)
    )
    mhalf = np.full((2, 1), -0.5, np.float32)

    in_maps = []
    for i in range(n_cores):
        sl = slice(i * npc, (i + 1) * npc)
        in_maps.append(
            {
                "kt": np.ascontiguousarray(K_T[:, sl]),
                "pxt": np.ascontiguousarray(all_pixels[sl].T.astype(np.float32)),
                "b2": b2,
                "alb": alb,
                "bre": bre,
                "mhalf": mhalf,
            }
        )
    return in_maps


_PROGRAM_CACHE = {}


def _get_program():
    key = (NPC, N_CEN, N_G, PIX_BLK, ARG_W)
    if key not in _PROGRAM_CACHE:
        _PROGRAM_CACHE[key] = _build_program(NPC, N_CEN, N_G, PIX_BLK, ARG_W)
    return _PROGRAM_CACHE[key]


def run(inputs, trace=False, trace_kwargs=None):
    """Run on 8 NeuronCores. Returns (full_output [N_PIX, 1], BassKernelResults)."""
    from concourse.bass_utils import run_bass_kernel_spmd

    nc = _get_program()
    in_maps = _prep_inputs(
        inputs["betas"],
        inputs["K_def"],
        inputs["all_pixels"],
        inputs["all_p_centers"],
        inputs["alphas"],
        NPC,
        N_G,
    )
    kwargs = {}
    if trace:
        kwargs["trace"] = True
        if trace_kwargs:
            kwargs["trace_kwargs"] = trace_kwargs
    res = run_bass_kernel_spmd(nc, in_maps, core_ids=list(range(N_CORES)), **kwargs)
    outs = [res.results[i]["out"] for i in range(N_CORES)]
    full = np.concatenate([np.asarray(o).T.reshape(-1) for o in outs])
    return full.reshape(N_PIX, 1).astype(np.float32), res


def kernel(betas, K_def, all_pixels, all_p_centers, alphas):
    out, _ = run(
        {
            "betas": betas,
            "K_def": K_def,
            "all_pixels": all_pixels,
            "all_p_centers": all_p_centers,
            "alphas": alphas,
        }
    )
    return out

